# revision 3
# baseline (speedup 1.0000x reference)
"""Causal self-attention on 8 trn2 NeuronCores.

Sharding: tensor-parallel over heads. Core c computes Q/K/V and attention
for heads {2c, 2c+1} over all batches (column-parallel W_q/W_k/W_v slices),
then an 8-rank AllToAll redistributes the per-head attention outputs so
each core runs the full output projection (row-parallel contraction over
all 16 heads' features) for its 1/8 chunk of the (B*L) rows.

Layout notes (per core):
 - All matmul operands are bf16; accumulation is fp32 in PSUM.
 - Q/K are produced transposed: QT/KT [128 part = 2 heads x 64 hd, B*L].
 - Scores are computed transposed: scoresT [k part, q free], so softmax's
   key-padding bias is a per-partition activation bias and the probs tile
   feeds the P@V matmul directly as the moving operand (no transpose).
 - Softmax skips max-subtraction (scores are O(1) for this input dist);
   denominators come from a ones-column appended to V (M=65 matmuls).
 - Causal masking: fully-masked key blocks are skipped structurally;
   diagonal blocks are multiplied by a precomputed 0/1 mask after exp.
"""

import numpy as np
import ml_dtypes

import concourse.bass as bass
import concourse.mybir as mybir
import concourse.tile as tile
from concourse import bacc
from concourse.bass_utils import run_bass_kernel_spmd

B, L, D, H, HD = 4, 2048, 1024, 16, 64
NCORES = 8
DL = 128              # local feature dim: 2 heads * 64
BL = B * L            # 8192
CHUNK = BL // NCORES  # 1024 output rows per core
SCALE = HD ** -0.5
NEG = -1e9

QT = 512              # query tile (free dim)
KB = 128              # key block (partition dim)
NQT = L // QT         # 4 q-tiles per batch
NKB = L // KB         # 16 k-blocks per batch
ND = D // 128         # 8 d_model partition tiles

FP32 = mybir.dt.float32
BF16 = mybir.dt.bfloat16
EXP = mybir.ActivationFunctionType.Exp

TRACE = False
LAST_EXEC_NS = None
_CACHED_NC = None


def build_program():
    nc = bacc.Bacc("TRN2", target_bir_lowering=False, debug=False,
                   num_devices=NCORES)
    xT = nc.dram_tensor("xT", [D, BL], BF16, kind="ExternalInput").ap()
    wq_t = nc.dram_tensor("wq_t", [D, DL], BF16, kind="ExternalInput").ap()
    wk_t = nc.dram_tensor("wk_t", [D, DL], BF16, kind="ExternalInput").ap()
    wv_t = nc.dram_tensor("wv_t", [D, DL], BF16, kind="ExternalInput").ap()
    wo_t = nc.dram_tensor("wo_t", [D, D], BF16, kind="ExternalInput").ap()
    bq_r = nc.dram_tensor("bq_r", [1, DL], BF16, kind="ExternalInput").ap()
    bk_r = nc.dram_tensor("bk_r", [1, DL], BF16, kind="ExternalInput").ap()
    bv_r = nc.dram_tensor("bv_r", [1, DL], BF16, kind="ExternalInput").ap()
    bo_r = nc.dram_tensor("bo_r", [1, D], BF16, kind="ExternalInput").ap()
    pad_b = nc.dram_tensor("pad_b", [KB, B * NKB], FP32, kind="ExternalInput").ap()
    cmask = nc.dram_tensor("cmask", [KB, 4 * QT], BF16, kind="ExternalInput").ap()
    out_chunk = nc.dram_tensor("out_chunk", [CHUNK, D], FP32,
                               kind="ExternalOutput").ap()

    with tile.TileContext(nc) as tc:
        with tc.tile_pool(name="persist", bufs=1) as persist, \
             tc.tile_pool(name="xpool", bufs=3) as xpool, \
             tc.tile_pool(name="probs", bufs=6) as probs, \
             tc.tile_pool(name="small", bufs=4) as small, \
             tc.tile_pool(name="opool", bufs=3) as opool, \
             tc.tile_pool(name="psum", bufs=2, space="PSUM") as psum, \
             tc.tile_pool(name="dram", bufs=1, space="DRAM") as dram, \
             tc.tile_pool(name="dram2", bufs=4, space="DRAM") as dram2:

            # ---- constants / weights into SBUF ----
            wq_sb = persist.tile([128, ND, 128], BF16)
            wk_sb = persist.tile([128, ND, 128], BF16)
            wv_sb = persist.tile([128, ND, 128], BF16)
            nc.sync.dma_start(out=wq_sb, in_=wq_t.rearrange("(t p) m -> p t m", p=128))
            nc.sync.dma_start(out=wk_sb, in_=wk_t.rearrange("(t p) m -> p t m", p=128))
            nc.sync.dma_start(out=wv_sb, in_=wv_t.rearrange("(t p) m -> p t m", p=128))
            wo_sb = persist.tile([128, ND, D], BF16)
            nc.sync.dma_start(out=wo_sb, in_=wo_t.rearrange("(t p) m -> p t m", p=128))
            cmask_sb = persist.tile([KB, 4 * QT], BF16)
            nc.sync.dma_start(out=cmask_sb, in_=cmask)
            pad_sb = persist.tile([KB, B * NKB], FP32)
            nc.sync.dma_start(out=pad_sb, in_=pad_b)
            bq_sb = persist.tile([1, DL], BF16)
            bk_sb = persist.tile([1, DL], BF16)
            bv_sb = persist.tile([1, DL], BF16)
            bo_sb = persist.tile([1, D], BF16)
            nc.sync.dma_start(out=bq_sb, in_=bq_r)
            nc.sync.dma_start(out=bk_sb, in_=bk_r)
            nc.sync.dma_start(out=bv_sb, in_=bv_r)
            nc.sync.dma_start(out=bo_sb, in_=bo_r)
            ones_sb = persist.tile([1, QT], BF16)
            nc.vector.memset(ones_sb, 1.0)

            # ---- persistent activations ----
            QT_sb = persist.tile([128, BL], BF16)       # [2h x 64, l]
            KT_sb = persist.tile([128, BL], BF16)
            V_sb = persist.tile([128, B * NKB, 130], BF16)  # [k, ktile, VA|1|VB|1]
            nc.vector.memset(V_sb, 1.0)                 # pre-set ones columns
            att_sb = persist.tile([64, 2 * BL], BF16)   # head h at cols h*BL
            gath_sb = persist.tile([128, NCORES, CHUNK], BF16)

            # ---- phase 1: QKV projections ----
            nlc = BL // QT
            for lc in range(nlc):
                xt = xpool.tile([128, ND, QT], BF16, tag="xt")
                nc.sync.dma_start(
                    out=xt,
                    in_=xT[:, QT * lc:QT * (lc + 1)].rearrange(
                        "(t p) l -> p t l", p=128))
                ps_q = psum.tile([128, QT], FP32, tag="psA")
                ps_k = psum.tile([128, QT], FP32, tag="psB")
                ps_v = psum.tile([128, QT], FP32, tag="psC")
                for dt in range(ND):
                    nc.tensor.matmul(ps_q, lhsT=wq_sb[:, dt, :], rhs=xt[:, dt, :],
                                     start=(dt == 0), stop=False)
                    nc.tensor.matmul(ps_k, lhsT=wk_sb[:, dt, :], rhs=xt[:, dt, :],
                                     start=(dt == 0), stop=False)
                nc.tensor.matmul(ps_q, lhsT=bq_sb, rhs=ones_sb,
                                 start=False, stop=True)
                nc.tensor.matmul(ps_k, lhsT=bk_sb, rhs=ones_sb,
                                 start=False, stop=True)
                for vs in range(QT // KB):
                    for dt in range(ND):
                        nc.tensor.matmul(ps_v[:, KB * vs:KB * (vs + 1)],
                                         lhsT=xt[:, dt, KB * vs:KB * (vs + 1)],
                                         rhs=wv_sb[:, dt, :],
                                         start=(dt == 0), stop=False)
                    nc.tensor.matmul(ps_v[:, KB * vs:KB * (vs + 1)],
                                     lhsT=ones_sb[:, 0:KB], rhs=bv_sb,
                                     start=False, stop=True)
                nc.vector.tensor_copy(QT_sb[:, QT * lc:QT * (lc + 1)], ps_q)
                nc.vector.tensor_copy(KT_sb[:, QT * lc:QT * (lc + 1)], ps_k)
                for vs in range(QT // KB):
                    kt = (QT // KB) * lc + vs
                    nc.vector.tensor_copy(V_sb[:, kt, 0:64],
                                          ps_v[:, KB * vs:KB * vs + 64])
                    nc.vector.tensor_copy(V_sb[:, kt, 65:129],
                                          ps_v[:, KB * vs + 64:KB * vs + 128])

            # ---- phase 2: attention (2 heads, transposed softmax) ----
            for b in range(B):
                for qt in range(NQT):
                    q0 = L * b + QT * qt
                    nkb = (QT // KB) * (qt + 1)
                    pv_a = psum.tile([65, QT], FP32, tag="psC")
                    pv_b = psum.tile([65, QT], FP32, tag="psD")
                    for j in range(nkb):
                        k0 = L * b + KB * j
                        kt = NKB * b + j
                        ps_sa = psum.tile([128, QT], FP32, tag="psA")
                        ps_sb2 = psum.tile([128, QT], FP32, tag="psB")
                        nc.tensor.matmul(ps_sa, lhsT=KT_sb[0:64, k0:k0 + KB],
                                         rhs=QT_sb[0:64, q0:q0 + QT],
                                         start=True, stop=True)
                        nc.tensor.matmul(ps_sb2, lhsT=KT_sb[64:128, k0:k0 + KB],
                                         rhs=QT_sb[64:128, q0:q0 + QT],
                                         start=True, stop=True)
                        pa = probs.tile([128, QT], BF16, tag="pa")
                        pb = probs.tile([128, QT], BF16, tag="pb")
                        bias_ap = pad_sb[:, kt:kt + 1]
                        nc.scalar.activation(pa, ps_sa, EXP, bias=bias_ap,
                                             scale=SCALE)
                        nc.scalar.activation(pb, ps_sb2, EXP, bias=bias_ap,
                                             scale=SCALE)
                        o = j - (QT // KB) * qt
                        if o >= 0:  # diagonal block: apply causal 0/1 mask
                            nc.vector.tensor_mul(pa, pa,
                                                 cmask_sb[:, QT * o:QT * (o + 1)])
                            nc.vector.tensor_mul(pb, pb,
                                                 cmask_sb[:, QT * o:QT * (o + 1)])
                        nc.tensor.matmul(pv_a, lhsT=V_sb[:, kt, 0:65], rhs=pa,
                                         start=(j == 0), stop=(j == nkb - 1))
                        nc.tensor.matmul(pv_b, lhsT=V_sb[:, kt, 65:130], rhs=pb,
                                         start=(j == 0), stop=(j == nkb - 1))
                    for h, pv in ((0, pv_a), (1, pv_b)):
                        rec = small.tile([1, QT], FP32, tag="rec")
                        nc.vector.reciprocal(rec, pv[64:65, :])
                        rec_dr = dram2.tile([1, QT], FP32, tag="rec_dr")
                        nc.sync.dma_start(out=rec_dr, in_=rec)
                        bc = small.tile([64, QT], FP32, tag="bc")
                        nc.sync.dma_start(out=bc,
                                          in_=rec_dr.to_broadcast([64, QT]))
                        nc.vector.tensor_mul(
                            att_sb[:, BL * h + q0:BL * h + q0 + QT],
                            pv[0:64, :], bc)

            # ---- phase 3: AllToAll over attention outputs ----
            a2a_in = dram.tile([NCORES * 128, CHUNK], BF16)
            a2a_out = dram.tile([NCORES * 128, CHUNK], BF16)
            for j in range(NCORES):
                nc.sync.dma_start(
                    out=a2a_in[128 * j:128 * j + 64, :],
                    in_=att_sb[:, CHUNK * j:CHUNK * (j + 1)])
                nc.sync.dma_start(
                    out=a2a_in[128 * j + 64:128 * (j + 1), :],
                    in_=att_sb[:, BL + CHUNK * j:BL + CHUNK * (j + 1)])
            nc.gpsimd.collective_compute(
                "AllToAll", mybir.AluOpType.bypass,
                replica_groups=[list(range(NCORES))],
                ins=[a2a_in.opt()], outs=[a2a_out.opt()])
            for j in range(NCORES):
                nc.sync.dma_start(out=gath_sb[:, j, :],
                                  in_=a2a_out[128 * j:128 * (j + 1), :])

            # ---- phase 4: output projection for my row chunk ----
            for lt in range(CHUNK // 128):
                for nt in range(D // QT):
                    ps_o = psum.tile([128, QT], FP32, tag="psA")
                    for dvt in range(ND):
                        nc.tensor.matmul(
                            ps_o,
                            lhsT=gath_sb[:, dvt, 128 * lt:128 * (lt + 1)],
                            rhs=wo_sb[:, dvt, QT * nt:QT * (nt + 1)],
                            start=(dvt == 0), stop=False)
                    nc.tensor.matmul(ps_o, lhsT=ones_sb[:, 0:128],
                                     rhs=bo_sb[:, QT * nt:QT * (nt + 1)],
                                     start=False, stop=True)
                    ot = opool.tile([128, QT], FP32, tag="ot")
                    nc.vector.tensor_copy(ot, ps_o)
                    nc.sync.dma_start(
                        out=out_chunk[128 * lt:128 * (lt + 1),
                                      QT * nt:QT * (nt + 1)],
                        in_=ot)

    nc.compile()
    return nc


def kernel(x, mask, W_q, b_q, W_k, b_k, W_v, b_v, W_o, b_o):
    global _CACHED_NC, LAST_EXEC_NS
    bf16 = ml_dtypes.bfloat16
    x = np.asarray(x, np.float32)
    mask = np.asarray(mask)

    xT = np.ascontiguousarray(x.reshape(BL, D).T).astype(bf16)
    wo_t = np.ascontiguousarray(np.asarray(W_o, np.float32).T).astype(bf16)
    bo = np.asarray(b_o, np.float32).reshape(1, D).astype(bf16)
    pb = np.where(mask != 0, 0.0, NEG).astype(np.float32)        # [B, L]
    pad = np.ascontiguousarray(
        pb.reshape(B, NKB, KB).transpose(2, 0, 1).reshape(KB, B * NKB))
    kp = np.arange(KB)[:, None, None]
    oo = np.arange(4)[None, :, None]
    qf = np.arange(QT)[None, None, :]
    cm = (qf >= oo * KB + kp).astype(np.float32).reshape(KB, 4 * QT).astype(bf16)

    in_maps = []
    for c in range(NCORES):
        sl = slice(DL * c, DL * (c + 1))
        in_maps.append({
            "xT": xT, "wo_t": wo_t, "bo_r": bo, "pad_b": pad, "cmask": cm,
            "wq_t": np.ascontiguousarray(
                np.asarray(W_q, np.float32)[sl].T).astype(bf16),
            "wk_t": np.ascontiguousarray(
                np.asarray(W_k, np.float32)[sl].T).astype(bf16),
            "wv_t": np.ascontiguousarray(
                np.asarray(W_v, np.float32)[sl].T).astype(bf16),
            "bq_r": np.asarray(b_q, np.float32)[sl].reshape(1, DL).astype(bf16),
            "bk_r": np.asarray(b_k, np.float32)[sl].reshape(1, DL).astype(bf16),
            "bv_r": np.asarray(b_v, np.float32)[sl].reshape(1, DL).astype(bf16),
        })

    if _CACHED_NC is None:
        _CACHED_NC = build_program()
    res = run_bass_kernel_spmd(_CACHED_NC, in_maps, list(range(NCORES)),
                               trace=TRACE)
    LAST_EXEC_NS = res.exec_time_ns
    out = np.concatenate(
        [res.results[c]["out_chunk"] for c in range(NCORES)], axis=0)
    return np.ascontiguousarray(out.reshape(B, L, D))


# revision 12
# speedup vs baseline: 16518.1017x; 16518.1017x over previous
"""Causal self-attention on 8 trn2 NeuronCores.

Sharding: tensor-parallel over heads. Core c computes Q/K/V and attention
for heads {2c, 2c+1} over all batches (column-parallel W_q/W_k/W_v slices),
then an 8-rank AllToAll redistributes the per-head attention outputs so
each core runs the full output projection (row-parallel contraction over
all 16 heads' features) for its 1/8 chunk of the (B*L) rows.

Layout notes (per core):
 - All matmul operands are bf16; accumulation is fp32 in PSUM.
 - Q/K are produced transposed: QT/KT [128 part = 2 heads x 64 hd, B*L].
 - Scores are computed transposed: scoresT [k part, q free], so softmax's
   key-padding bias is a per-partition activation bias and the probs tile
   feeds the P@V matmul directly as the moving operand (no transpose).
 - Softmax skips max-subtraction (scores are O(1) for this input dist);
   denominators come from a ones-column appended to V (M=65 matmuls).
 - Causal masking: fully-masked key blocks are skipped structurally;
   diagonal blocks are multiplied by a precomputed 0/1 mask after exp.
"""

import numpy as np
import ml_dtypes

import concourse.bass as bass
import concourse.mybir as mybir
import concourse.tile as tile
from concourse import bacc
from concourse.bass_utils import run_bass_kernel_spmd

B, L, D, H, HD = 4, 2048, 1024, 16, 64
NCORES = 8
DL = 128              # local feature dim: 2 heads * 64
BL = B * L            # 8192
CHUNK = BL // NCORES  # 1024 output rows per core
SCALE = HD ** -0.5
NEG = -1e9

QT = 512              # query tile (free dim)
KB = 128              # key block (partition dim)
NQT = L // QT         # 4 q-tiles per batch
NKB = L // KB         # 16 k-blocks per batch
ND = D // 128         # 8 d_model partition tiles

FP32 = mybir.dt.float32
BF16 = mybir.dt.bfloat16
EXP = mybir.ActivationFunctionType.Exp

TRACE = False
LAST_EXEC_NS = None
_CACHED_NC = None
_SIM_MODE = False   # replace the collective with a local DMA; 1 device


def build_program():
    nc = bacc.Bacc("TRN2", target_bir_lowering=False, debug=False,
                   num_devices=(1 if _SIM_MODE else NCORES))
    xT = nc.dram_tensor("xT", [D, BL], BF16, kind="ExternalInput").ap()
    wq_t = nc.dram_tensor("wq_t", [D, DL], BF16, kind="ExternalInput").ap()
    wk_t = nc.dram_tensor("wk_t", [D, DL], BF16, kind="ExternalInput").ap()
    wv_t = nc.dram_tensor("wv_t", [D, DL], BF16, kind="ExternalInput").ap()
    wo_t = nc.dram_tensor("wo_t", [D, D], BF16, kind="ExternalInput").ap()
    bq_r = nc.dram_tensor("bq_r", [1, DL], BF16, kind="ExternalInput").ap()
    bk_r = nc.dram_tensor("bk_r", [1, DL], BF16, kind="ExternalInput").ap()
    bv_r = nc.dram_tensor("bv_r", [1, DL], BF16, kind="ExternalInput").ap()
    bo_r = nc.dram_tensor("bo_r", [1, D], BF16, kind="ExternalInput").ap()
    pad_b = nc.dram_tensor("pad_b", [KB, B * NKB], FP32, kind="ExternalInput").ap()
    cmask = nc.dram_tensor("cmask", [KB, KB], BF16, kind="ExternalInput").ap()
    out_chunk = nc.dram_tensor("out_chunk", [CHUNK, D], FP32,
                               kind="ExternalOutput").ap()

    with tile.TileContext(nc) as tc:
        with tc.tile_pool(name="persist", bufs=1) as persist, \
             tc.tile_pool(name="xpool", bufs=3) as xpool, \
             tc.tile_pool(name="probs", bufs=6) as probs, \
             tc.tile_pool(name="small", bufs=4) as small, \
             tc.tile_pool(name="opool", bufs=3) as opool, \
             tc.tile_pool(name="psum", bufs=2, space="PSUM") as psum, \
             tc.tile_pool(name="dram", bufs=1, space="DRAM") as dram, \
             tc.tile_pool(name="dram2", bufs=4, space="DRAM") as dram2:

            # ---- constants / weights into SBUF ----
            wq_sb = persist.tile([128, ND, 128], BF16)
            wk_sb = persist.tile([128, ND, 128], BF16)
            wv_sb = persist.tile([128, ND, 128], BF16)
            nc.sync.dma_start(out=wq_sb, in_=wq_t.rearrange("(t p) m -> p t m", p=128))
            nc.sync.dma_start(out=wk_sb, in_=wk_t.rearrange("(t p) m -> p t m", p=128))
            nc.sync.dma_start(out=wv_sb, in_=wv_t.rearrange("(t p) m -> p t m", p=128))
            wo_sb = persist.tile([128, ND, D], BF16)
            nc.sync.dma_start(out=wo_sb, in_=wo_t.rearrange("(t p) m -> p t m", p=128))
            cmask_sb = persist.tile([KB, KB], BF16)
            nc.sync.dma_start(out=cmask_sb, in_=cmask)
            pad_sb = persist.tile([KB, B * NKB], FP32)
            nc.sync.dma_start(out=pad_sb, in_=pad_b)
            bq_sb = persist.tile([1, DL], BF16)
            bk_sb = persist.tile([1, DL], BF16)
            bv_sb = persist.tile([1, DL], BF16)
            bo_sb = persist.tile([1, D], BF16)
            nc.sync.dma_start(out=bq_sb, in_=bq_r)
            nc.sync.dma_start(out=bk_sb, in_=bk_r)
            nc.sync.dma_start(out=bv_sb, in_=bv_r)
            nc.sync.dma_start(out=bo_sb, in_=bo_r)
            ones_sb = persist.tile([1, QT], BF16)
            nc.vector.memset(ones_sb, 1.0)

            # ---- persistent activations ----
            QT_sb = persist.tile([128, BL], BF16)       # [2h x 64, l]
            KT_sb = persist.tile([128, BL], BF16)
            V_sb = persist.tile([128, B * NKB, 130], BF16)  # [k, ktile, VA|1|VB|1]
            nc.vector.memset(V_sb, 1.0)                 # pre-set ones columns
            att_sb = persist.tile([64, 2 * BL], BF16)   # head h at cols h*BL

            # ---- phase 1: QKV projections ----
            nlc = BL // QT
            for lc in range(nlc):
                xt = xpool.tile([128, ND, QT], BF16, tag="xt")
                nc.sync.dma_start(
                    out=xt,
                    in_=xT[:, QT * lc:QT * (lc + 1)].rearrange(
                        "(t p) l -> p t l", p=128))
                ps_q = psum.tile([128, QT], FP32, tag="psA")
                ps_k = psum.tile([128, QT], FP32, tag="psB")
                ps_v = psum.tile([128, QT], FP32, tag="psC")
                for dt in range(ND):
                    nc.tensor.matmul(ps_q, lhsT=wq_sb[:, dt, :], rhs=xt[:, dt, :],
                                     start=(dt == 0), stop=False)
                    nc.tensor.matmul(ps_k, lhsT=wk_sb[:, dt, :], rhs=xt[:, dt, :],
                                     start=(dt == 0), stop=False)
                nc.tensor.matmul(ps_q, lhsT=bq_sb, rhs=ones_sb,
                                 start=False, stop=True)
                nc.tensor.matmul(ps_k, lhsT=bk_sb, rhs=ones_sb,
                                 start=False, stop=True)
                for vs in range(QT // KB):
                    for dt in range(ND):
                        nc.tensor.matmul(ps_v[:, KB * vs:KB * (vs + 1)],
                                         lhsT=xt[:, dt, KB * vs:KB * (vs + 1)],
                                         rhs=wv_sb[:, dt, :],
                                         start=(dt == 0), stop=False)
                    nc.tensor.matmul(ps_v[:, KB * vs:KB * (vs + 1)],
                                     lhsT=ones_sb[:, 0:KB], rhs=bv_sb,
                                     start=False, stop=True)
                nc.vector.tensor_copy(QT_sb[:, QT * lc:QT * (lc + 1)], ps_q)
                nc.vector.tensor_copy(KT_sb[:, QT * lc:QT * (lc + 1)], ps_k)
                for vs in range(QT // KB):
                    kt = (QT // KB) * lc + vs
                    nc.vector.tensor_copy(V_sb[:, kt, 0:64],
                                          ps_v[:, KB * vs:KB * vs + 64])
                    nc.vector.tensor_copy(V_sb[:, kt, 65:129],
                                          ps_v[:, KB * vs + 64:KB * vs + 128])

            # ---- phase 2: attention (2 heads, transposed softmax) ----
            for b in range(B):
                for qt in range(NQT):
                    q0 = L * b + QT * qt
                    nkb = (QT // KB) * (qt + 1)
                    pv_a = psum.tile([65, QT], FP32, tag="psC")
                    pv_b = psum.tile([65, QT], FP32, tag="psD")
                    for j in range(nkb):
                        k0 = L * b + KB * j
                        kt = NKB * b + j
                        ps_sa = psum.tile([128, QT], FP32, tag="psA")
                        ps_sb2 = psum.tile([128, QT], FP32, tag="psB")
                        nc.tensor.matmul(ps_sa, lhsT=KT_sb[0:64, k0:k0 + KB],
                                         rhs=QT_sb[0:64, q0:q0 + QT],
                                         start=True, stop=True)
                        nc.tensor.matmul(ps_sb2, lhsT=KT_sb[64:128, k0:k0 + KB],
                                         rhs=QT_sb[64:128, q0:q0 + QT],
                                         start=True, stop=True)
                        pa = probs.tile([128, QT], BF16, tag="pa")
                        pb = probs.tile([128, QT], BF16, tag="pb")
                        bias_ap = pad_sb[:, kt:kt + 1]
                        o = j - (QT // KB) * qt
                        if o < 0:  # fully below the diagonal: plain exp
                            nc.scalar.activation(pa, ps_sa, EXP, bias=bias_ap,
                                                 scale=SCALE)
                            nc.scalar.activation(pb, ps_sb2, EXP, bias=bias_ap,
                                                 scale=SCALE)
                        else:
                            # diagonal block: cols [0, 128o) are fully masked,
                            # [128o, 128o+128) is triangular, rest fully valid
                            c0 = KB * o
                            for p, ps in ((pa, ps_sa), (pb, ps_sb2)):
                                if o > 0:
                                    nc.vector.memset(p[:, 0:c0], 0.0)
                                nc.scalar.activation(p[:, c0:QT], ps[:, c0:QT],
                                                     EXP, bias=bias_ap,
                                                     scale=SCALE)
                                nc.vector.tensor_mul(p[:, c0:c0 + KB],
                                                     p[:, c0:c0 + KB], cmask_sb)
                        nc.tensor.matmul(pv_a, lhsT=V_sb[:, kt, 0:65], rhs=pa,
                                         start=(j == 0), stop=(j == nkb - 1))
                        nc.tensor.matmul(pv_b, lhsT=V_sb[:, kt, 65:130], rhs=pb,
                                         start=(j == 0), stop=(j == nkb - 1))
                    for h, pv in ((0, pv_a), (1, pv_b)):
                        rec = small.tile([1, QT], FP32, tag="rec")
                        nc.vector.reciprocal(rec, pv[64:65, :])
                        rec_dr = dram2.tile([1, QT], FP32, tag="rec_dr")
                        nc.sync.dma_start(out=rec_dr, in_=rec)
                        bc = small.tile([64, QT], FP32, tag="bc")
                        nc.sync.dma_start(out=bc,
                                          in_=rec_dr.to_broadcast([64, QT]))
                        nc.vector.tensor_mul(
                            att_sb[:, BL * h + q0:BL * h + q0 + QT],
                            pv[0:64, :], bc)

            # ---- phases 3+4: two half AllToAlls, each followed by the
            # output projection for its 512-row block. Core c's output rows
            # are global 512-row blocks {c, 8+c}; the first A2A (batches
            # 0-1) overlaps the attention compute of batches 2-3.
            HB = 512  # half-block rows per core per A2A
            for p in range(2):
                a2a_in = dram.tile([NCORES * 128, HB], BF16, tag=f"a2a_in{p}",
                                   name=f"a2a_in{p}")
                a2a_out = dram.tile([NCORES * 128, HB], BF16, tag=f"a2a_out{p}",
                                    name=f"a2a_out{p}")
                base = p * NCORES * HB  # att col offset of this half
                for j in range(NCORES):
                    nc.sync.dma_start(
                        out=a2a_in[128 * j:128 * j + 64, :],
                        in_=att_sb[:, base + HB * j:base + HB * (j + 1)])
                    nc.sync.dma_start(
                        out=a2a_in[128 * j + 64:128 * (j + 1), :],
                        in_=att_sb[:, BL + base + HB * j:
                                   BL + base + HB * (j + 1)])
                if _SIM_MODE:
                    nc.sync.dma_start(out=a2a_out, in_=a2a_in)
                else:
                    nc.gpsimd.collective_compute(
                        "AllToAll", mybir.AluOpType.bypass,
                        replica_groups=[list(range(NCORES))],
                        ins=[a2a_in.opt()], outs=[a2a_out.opt()])
                gath = persist.tile([128, NCORES, HB], BF16, tag=f"gath{p}",
                                    name=f"gath{p}")
                for j in range(NCORES):
                    nc.sync.dma_start(out=gath[:, j, :],
                                      in_=a2a_out[128 * j:128 * (j + 1), :])
                for lt in range(HB // 128):
                    for nt in range(D // QT):
                        ps_o = psum.tile([128, QT], FP32, tag="psA")
                        for dvt in range(ND):
                            nc.tensor.matmul(
                                ps_o,
                                lhsT=gath[:, dvt, 128 * lt:128 * (lt + 1)],
                                rhs=wo_sb[:, dvt, QT * nt:QT * (nt + 1)],
                                start=(dvt == 0), stop=False)
                        nc.tensor.matmul(ps_o, lhsT=ones_sb[:, 0:128],
                                         rhs=bo_sb[:, QT * nt:QT * (nt + 1)],
                                         start=False, stop=True)
                        ot = opool.tile([128, QT], FP32, tag="ot")
                        nc.vector.tensor_copy(ot, ps_o)
                        nc.sync.dma_start(
                            out=out_chunk[HB * p + 128 * lt:
                                          HB * p + 128 * (lt + 1),
                                          QT * nt:QT * (nt + 1)],
                            in_=ot)

    nc.compile()
    return nc


def kernel(x, mask, W_q, b_q, W_k, b_k, W_v, b_v, W_o, b_o):
    global _CACHED_NC, LAST_EXEC_NS
    bf16 = ml_dtypes.bfloat16
    x = np.asarray(x, np.float32)
    mask = np.asarray(mask)

    xT = np.ascontiguousarray(x.reshape(BL, D).T).astype(bf16)
    wo_t = np.ascontiguousarray(np.asarray(W_o, np.float32).T).astype(bf16)
    bo = np.asarray(b_o, np.float32).reshape(1, D).astype(bf16)
    pb = np.where(mask != 0, 0.0, NEG).astype(np.float32)        # [B, L]
    pad = np.ascontiguousarray(
        pb.reshape(B, NKB, KB).transpose(2, 0, 1).reshape(KB, B * NKB))
    kp = np.arange(KB)[:, None]
    qs = np.arange(KB)[None, :]
    cm = (qs >= kp).astype(np.float32).astype(bf16)   # [128, 128] triangle

    in_maps = []
    for c in range(NCORES):
        sl = slice(DL * c, DL * (c + 1))
        in_maps.append({
            "xT": xT, "wo_t": wo_t, "bo_r": bo, "pad_b": pad, "cmask": cm,
            "wq_t": np.ascontiguousarray(
                np.asarray(W_q, np.float32)[sl].T).astype(bf16),
            "wk_t": np.ascontiguousarray(
                np.asarray(W_k, np.float32)[sl].T).astype(bf16),
            "wv_t": np.ascontiguousarray(
                np.asarray(W_v, np.float32)[sl].T).astype(bf16),
            "bq_r": np.asarray(b_q, np.float32)[sl].reshape(1, DL).astype(bf16),
            "bk_r": np.asarray(b_k, np.float32)[sl].reshape(1, DL).astype(bf16),
            "bv_r": np.asarray(b_v, np.float32)[sl].reshape(1, DL).astype(bf16),
        })

    if _CACHED_NC is None:
        _CACHED_NC = build_program()
    res = run_bass_kernel_spmd(_CACHED_NC, in_maps, list(range(NCORES)),
                               trace=TRACE)
    LAST_EXEC_NS = res.exec_time_ns
    # core c's out_chunk rows [0:512] are global rows [512c:512c+512],
    # rows [512:1024] are global rows [4096+512c : 4096+512c+512]
    out = np.empty((BL, D), np.float32)
    for c in range(NCORES):
        oc = res.results[c]["out_chunk"]
        out[512 * c:512 * (c + 1)] = oc[0:512]
        out[BL // 2 + 512 * c:BL // 2 + 512 * (c + 1)] = oc[512:1024]
    return np.ascontiguousarray(out.reshape(B, L, D))
